# revision 21
# baseline (speedup 1.0000x reference)
"""MoE layer (B=4, N=2048, C=1024, F=4096, E=8, top-2) on 8 trn2 NeuronCores.

Sharding: expert-parallel. The host computes the (tiny, ~0.1% of FLOPs)
router and dispatches each expert's tokens to one core as part of sharding;
each core runs its expert's full FFN  relu(xg @ w1.T + b1) @ w2.T + b2,
gated by the combine weight, over its gathered tokens.  The host combine
scatter-adds the per-expert results back into the full output.

Device kernel (per core, SPMD, identical program):
  inputs : xgT [1024, cap]  (gathered tokens, transposed)
           w1t [1024, 4096] (w1[e].T)   w2t [4096, 1024] (w2[e].T)
           b1r [128, 32]    (b1[e] partition-major)
           b2r [128, 1024]  (b2[e] replicated over partitions)
           wg  [128, cap/128] (combine weights, partition-major)
  output : yg  [cap, 1024]
Matmuls run as float32r (FP22 multiply, fp32 accumulate) — 4x the fp32 rate.
"""

import numpy as np

P = 128
C = 1024
F = 4096
E = 8
SCH = 384  # token chunk: 3 PSUM banks (x 2 C-halves) for y + 2 for h = 8


def _build(cap: int):
    import concourse.mybir as mybir
    from concourse import bacc
    from concourse.tile import TileContext

    f32 = mybir.dt.float32
    f32r = mybir.dt.float32r
    nS = cap // SCH
    nc = bacc.Bacc(None, target_bir_lowering=False)

    xgT = nc.dram_tensor("xgT", [C, cap], f32, kind="ExternalInput")
    w1t = nc.dram_tensor("w1t", [C, F], f32, kind="ExternalInput")
    w2t = nc.dram_tensor("w2t", [F, C], f32, kind="ExternalInput")
    b1r = nc.dram_tensor("b1r", [P, F // P], f32, kind="ExternalInput")
    b2r = nc.dram_tensor("b2r", [P, C], f32, kind="ExternalInput")
    wg = nc.dram_tensor("wg", [P, cap // P], f32, kind="ExternalInput")
    yg = nc.dram_tensor("yg", [cap, C], f32, kind="ExternalOutput")

    w1v = w1t.ap().rearrange("(co ci) f -> ci co f", ci=P)  # [128, 8, F]
    xgv = xgT.ap().rearrange("(co ci) n -> ci co n", ci=P)  # [128, 8, cap]

    with TileContext(nc) as tc:
        with (
            tc.tile_pool(name="consts", bufs=1) as consts,
            tc.tile_pool(name="wpool", bufs=4) as wpool,
            tc.tile_pool(name="xpool", bufs=2) as xpool,
            tc.tile_pool(name="hpool", bufs=3) as hpool,
            tc.tile_pool(name="ypool", bufs=3) as ypool,
            tc.tile_pool(name="psum_h", bufs=2, space="PSUM") as psum_h,
            tc.tile_pool(name="psum_y", bufs=1, space="PSUM") as psum_y,
        ):
            b1_sb = consts.tile([P, F // P], f32)
            nc.sync.dma_start(b1_sb[:], b1r[:, :])
            b2_sb = consts.tile([P, C], f32)
            nc.sync.dma_start(b2_sb[:], b2r[:, :])
            wg_sb = consts.tile([P, cap // P], f32)
            nc.sync.dma_start(wg_sb[:], wg[:, :])

            for s in range(nS):
                xg_s = xpool.tile([P, 8, SCH], f32r, tag="xg")
                nc.sync.dma_start(xg_s[:], xgv[:, :, s * SCH : (s + 1) * SCH].bitcast(f32r))

                yps = [
                    [
                        psum_y.tile(
                            [P, 512], f32, tag=f"y_{t}_{cc}", name=f"y_{t}_{cc}"
                        )
                        for cc in range(2)
                    ]
                    for t in range(3)
                ]

                for f in range(F // P):  # 32
                    w1c = wpool.tile([P, 8, P], f32r, tag="w1c")
                    nc.sync.dma_start(w1c[:], w1v[:, :, f * P : (f + 1) * P].bitcast(f32r))
                    w2c = wpool.tile([P, C], f32r, tag="w2c")
                    nc.sync.dma_start(w2c[:], w2t[f * P : (f + 1) * P, :].bitcast(f32r))

                    hps = psum_h.tile([P, SCH], f32, tag="h")
                    for c in range(8):
                        nc.tensor.matmul(
                            hps[:],
                            lhsT=w1c[:, c, :],
                            rhs=xg_s[:, c, :],
                            start=(c == 0),
                            stop=(c == 7),
                        )
                    hT = hpool.tile([P, SCH], f32r, tag="hT")
                    nc.scalar.activation(
                        hT[:],
                        hps[:],
                        mybir.ActivationFunctionType.Relu,
                        bias=b1_sb[:, f : f + 1],
                        scale=1.0,
                    )
                    for t in range(3):
                        for cc in range(2):
                            nc.tensor.matmul(
                                yps[t][cc][:],
                                lhsT=hT[:, t * P : (t + 1) * P],
                                rhs=w2c[:, cc * 512 : (cc + 1) * 512],
                                start=(f == 0),
                                stop=(f == F // P - 1),
                            )

                for t in range(3):
                    y_sb = ypool.tile([P, C], f32, tag="y_sb")
                    for cc in range(2):
                        sl = slice(cc * 512, (cc + 1) * 512)
                        nc.vector.tensor_add(y_sb[:, sl], yps[t][cc][:], b2_sb[:, sl])
                    yf = ypool.tile([P, C], f32, tag="yf")
                    nc.scalar.mul(yf[:], y_sb[:], wg_sb[:, s * 3 + t : s * 3 + t + 1])
                    nc.sync.dma_start(
                        yg[(s * 3 + t) * P : (s * 3 + t + 1) * P, :], yf[:]
                    )
    nc.compile()
    return nc




def _chunks(cap):
    sizes = [384] * (cap // 384)
    rem = cap - 384 * len(sizes)
    if rem:
        sizes.append(rem)  # runt chunk last: shortest possible drain tail
    return sizes


def _build_fast(cap: int):
    """Fast path (b1 == 0 and b2 == 0): inputs pre-gated and pre-tiled on host.

    All matmul operands are bf16 (quant error ~3e-3 << 2e-2 budget).  All
    weights stay resident in SBUF (128 KiB/partition), loaded once via
    per-fl tiles in consumption order; w1 issues on the gpsimd queue and w2
    on the vector queue so the sync queue (~600ns per dma_start) never
    serializes ahead of them.  Per-chunk y accumulates in PSUM across all
    32 f-tiles (6 banks y + 2 banks h), retired once per chunk.  A DVE
    memset feeds dummy matmuls at t~=0 so the PE HAM un-throttles before
    the first real matmul.
      inputs : xgf [cap*1024]  gated tokens bf16, per-chunk tiled [ci, co, n]
               w1p [32, 128, 8, 128]  w1.T tiled for mm1 lhsT (bf16)
               w2t [4096, 1024] (bf16)
      output : yg  [cap, 1024] f32
    """
    import concourse.mybir as mybir
    from concourse import bacc
    from concourse.tile import TileContext

    f32 = mybir.dt.float32
    bf16 = mybir.dt.bfloat16
    sizes = _chunks(cap)
    offs = [sum(sizes[:i]) for i in range(len(sizes))]
    NF = F // P  # 32
    NDUM = 24
    nc = bacc.Bacc(None, target_bir_lowering=False)

    xgf = nc.dram_tensor("xgf", [cap * C], bf16, kind="ExternalInput")
    w1p = nc.dram_tensor("w1p", [NF, P, 8, P], bf16, kind="ExternalInput")
    w2t = nc.dram_tensor("w2t", [F, C], bf16, kind="ExternalInput")
    yg = nc.dram_tensor("yg", [cap, C], bf16, kind="ExternalOutput")

    with TileContext(nc) as tc:
        with (
            tc.tile_pool(name="consts", bufs=1) as consts,
            tc.tile_pool(name="w1pool", bufs=NF) as w1pool,
            tc.tile_pool(name="w2pool", bufs=NF) as w2pool,
            tc.tile_pool(name="xhead", bufs=8) as xhead,
            tc.tile_pool(name="xpool", bufs=2) as xpool,
            tc.tile_pool(name="hpool", bufs=3) as hpool,
            tc.tile_pool(name="opool", bufs=3) as opool,
            tc.tile_pool(name="psum_h", bufs=2, space="PSUM") as psum_h,
            tc.tile_pool(name="psum_y", bufs=1, space="PSUM") as psum_y,
        ):
            warm = consts.tile([P, P], bf16)
            nc.vector.memset(warm[:], 0.0)
            wps = psum_h.tile([P, 384], f32, tag="h", name="wps")

            def dummy_mms(n):
                # dependency-free matmuls: fill PE gaps while the head DMAs
                # trickle in so the HAM activity window never resets
                for _ in range(n):
                    nc.tensor.matmul(
                        wps[:, :P], lhsT=warm[:], rhs=warm[:], start=True, stop=True
                    )

            dummy_mms(NDUM)

            def load_xg(s, split=False):
                sz = sizes[s]
                src = xgf[offs[s] * C : (offs[s] + sz) * C]
                v = src.rearrange("(ci co n) -> ci co n", ci=P, co=8)
                if split:  # head: per-co tiles so mm1(c) chases the DMA;
                    # issue on two queues (sync+scalar) to halve the
                    # ~650ns-per-descriptor serialization
                    ts = [
                        xhead.tile([P, sz], bf16, tag=f"xh{co}", name="xh")
                        for co in range(8)
                    ]
                    for co in range(8):
                        eng = nc.sync if co < 4 else nc.scalar
                        eng.dma_start(ts[co][:], v[:, co, :])
                    return [t[:] for t in ts]
                xg_s = xpool.tile([P, 8, sz], bf16, tag="xg", name="xg_s")
                nc.sync.dma_start(xg_s[:], v)
                return [xg_s[:, co, :] for co in range(8)]

            xg_cur = load_xg(0, split=True)
            w1f, w2f = [], []
            for fl in range(NF):
                t1 = w1pool.tile([P, 8, P], bf16, tag="w1f", name="w1f")
                nc.gpsimd.dma_start(t1[:], w1p[fl])
                w1f.append(t1)
                t2 = w2pool.tile([P, C], bf16, tag="w2f", name="w2f")
                nc.sync.dma_start(t2[:], w2t[fl * P : (fl + 1) * P, :])
                w2f.append(t2)

            for s, sz in enumerate(sizes):
                nt = (sz + P - 1) // P
                xg_s = xg_cur
                if s + 1 < len(sizes):
                    xg_cur = load_xg(s + 1)

                yps = [
                    psum_y.tile([P, C], f32, tag=f"y_{t}", name=f"y_{t}")
                    for t in range(nt)
                ]

                def mm2(fl, hT, yps=yps, nt=nt):
                    for t in range(nt):
                        for cc in range(2):
                            nc.tensor.matmul(
                                yps[t][:, cc * 512 : (cc + 1) * 512],
                                lhsT=hT[:, t * P : (t + 1) * P],
                                rhs=w2f[fl][:, cc * 512 : (cc + 1) * 512],
                                start=(fl == 0),
                                stop=(fl == NF - 1),
                            )

                # software pipeline: mm2 runs one fl behind mm1 (relu
                # latency covered by the next fl's mm1s)
                hT_prev = None
                for fl in range(NF):
                    hps = psum_h.tile([P, 384], f32, tag="h", name="hps")
                    if s == 0 and fl < 6:
                        # fill head DMA-pacing gaps so the HAM activity
                        # window never resets; the real mm1's start=True
                        # clears the region, so these are side-effect-free
                        for _ in range(6):
                            nc.tensor.matmul(
                                hps[:, :P],
                                lhsT=warm[:],
                                rhs=warm[:],
                                start=True,
                                stop=True,
                            )
                    for c in range(8):
                        nc.tensor.matmul(
                            hps[:, :sz],
                            lhsT=w1f[fl][:, c, :],
                            rhs=xg_s[c],
                            start=(c == 0),
                            stop=(c == 7),
                        )
                    hT = hpool.tile([P, 384], bf16, tag="hT", name="hT")
                    if fl == NF - 1:
                        # last fl: per-token-tile relu so mm2(t) can
                        # start as soon as its slice is ready
                        for t in range(nt):
                            tl = slice(t * P, min((t + 1) * P, sz))
                            nc.scalar.activation(
                                hT[:, tl],
                                hps[:, tl],
                                mybir.ActivationFunctionType.Relu,
                            )
                    else:
                        nc.scalar.activation(
                            hT[:, :sz],
                            hps[:, :sz],
                            mybir.ActivationFunctionType.Relu,
                        )
                    if hT_prev is not None:
                        mm2(fl - 1, hT_prev)
                    hT_prev = hT
                mm2(NF - 1, hT_prev)
                for t in range(nt):
                    yo = opool.tile([P, C], bf16, tag="yo", name="yo")
                    nc.vector.tensor_copy(yo[:], yps[t][:])
                    w0 = offs[s] // P + t
                    nc.sync.dma_start(yg[w0 * P : (w0 + 1) * P, :], yo[:])
    nc.compile()
    return nc


_CACHE = {}
_TRACE = False  # test harness sets True to capture an NTFF profile
_LAST_RES = None


def _get_nc(cap, fast):
    key = (cap, fast)
    if key not in _CACHE:
        _CACHE[key] = _build_fast(cap) if fast else _build(cap)
    return _CACHE[key]


def _route(x_flat, router_w):
    """Top-2 routing, float64 for stable selection. Returns idx/weights per expert."""
    logits = x_flat.astype(np.float64) @ router_w.astype(np.float64).T
    t = np.exp(logits - logits.max(-1, keepdims=True))
    p = t / t.sum(-1, keepdims=True)
    top2 = np.argsort(-p, axis=-1)[:, :2]
    pv = np.take_along_axis(p, top2, axis=-1)
    wn = pv / (pv.sum(-1, keepdims=True) + 1e-9)
    return top2, wn


def kernel(x, router_w, w1, b1, w2, b2):
    from concourse.bass_utils import run_bass_kernel_spmd

    Bx, Nx, Cx = x.shape
    x_flat = np.ascontiguousarray(x.reshape(-1, Cx))
    T = x_flat.shape[0]

    top2, wn = _route(x_flat, router_w)
    idxs, gates = [], []
    for e in range(E):
        sel = top2 == e
        we = np.where(sel, wn, 0.0).sum(-1)
        idx = np.nonzero(sel.any(-1))[0]
        idxs.append(idx)
        gates.append(we[idx].astype(np.float32))
    cap = max(len(i) for i in idxs)
    fastcap = ((cap + P - 1) // P) * P
    cap = ((cap + SCH - 1) // SCH) * SCH

    fast = bool(np.all(b1 == 0) and np.all(b2 == 0))
    if fast:
        cap = fastcap
    nc = _get_nc(cap, fast)

    in_maps = []
    for e in range(E):
        n_e = len(idxs[e])
        xg = np.zeros((cap, Cx), np.float32)
        xg[:n_e] = x_flat[idxs[e]]
        wg = np.zeros(cap, np.float32)
        wg[:n_e] = gates[e]
        if fast:
            import ml_dtypes

            bf16 = ml_dtypes.bfloat16
            xg *= wg[:, None]  # pre-gate: exact since b1 == 0 and wg >= 0
            sizes = _chunks(cap)
            blocks, off = [], 0
            for sz in sizes:
                blocks.append(
                    np.ascontiguousarray(
                        xg[off : off + sz].reshape(sz, 8, P).transpose(2, 1, 0)
                    )
                    .ravel()
                    .astype(bf16)
                )
                off += sz
            in_maps.append(
                {
                    "xgf": np.concatenate(blocks),
                    "w1p": np.ascontiguousarray(
                        w1[e].reshape(F // P, P, 8, P).transpose(0, 3, 2, 1)
                    ).astype(bf16),
                    "w2t": np.ascontiguousarray(w2[e].T).astype(bf16),
                }
            )
        else:
            in_maps.append(
                {
                    "xgT": np.ascontiguousarray(xg.T),
                    "w1t": np.ascontiguousarray(w1[e].T),
                    "w2t": np.ascontiguousarray(w2[e].T),
                    "b1r": np.ascontiguousarray(b1[e].reshape(F // P, P).T),
                    "b2r": np.ascontiguousarray(np.broadcast_to(b2[e], (P, Cx))),
                    "wg": np.ascontiguousarray(wg.reshape(cap // P, P).T),
                }
            )

    global _LAST_RES
    res = run_bass_kernel_spmd(nc, in_maps, core_ids=list(range(E)), trace=_TRACE)
    _LAST_RES = res

    out = np.zeros((T, Cx), np.float32)
    for e in range(E):
        n_e = len(idxs[e])
        out[idxs[e]] += res.results[e]["yg"][:n_e].astype(np.float32)
    return out.reshape(Bx, Nx, Cx)



# revision 30
# speedup vs baseline: 1.0088x; 1.0088x over previous
"""MoE layer (B=4, N=2048, C=1024, F=4096, E=8, top-2) on 8 trn2 NeuronCores.

Sharding: expert-parallel. The host computes the (tiny, ~0.1% of FLOPs)
router and dispatches each expert's tokens to one core as part of sharding;
each core runs its expert's full FFN  relu(xg @ w1.T + b1) @ w2.T + b2,
gated by the combine weight, over its gathered tokens.  The host combine
scatter-adds the per-expert results back into the full output.

Device kernel (per core, SPMD, identical program):
  inputs : xgT [1024, cap]  (gathered tokens, transposed)
           w1t [1024, 4096] (w1[e].T)   w2t [4096, 1024] (w2[e].T)
           b1r [128, 32]    (b1[e] partition-major)
           b2r [128, 1024]  (b2[e] replicated over partitions)
           wg  [128, cap/128] (combine weights, partition-major)
  output : yg  [cap, 1024]
Matmuls run as float32r (FP22 multiply, fp32 accumulate) — 4x the fp32 rate.
"""

import numpy as np

P = 128
C = 1024
F = 4096
E = 8
SCH = 384  # token chunk: 3 PSUM banks (x 2 C-halves) for y + 2 for h = 8


def _build(cap: int):
    import concourse.mybir as mybir
    from concourse import bacc
    from concourse.tile import TileContext

    f32 = mybir.dt.float32
    f32r = mybir.dt.float32r
    nS = cap // SCH
    nc = bacc.Bacc(None, target_bir_lowering=False)

    xgT = nc.dram_tensor("xgT", [C, cap], f32, kind="ExternalInput")
    w1t = nc.dram_tensor("w1t", [C, F], f32, kind="ExternalInput")
    w2t = nc.dram_tensor("w2t", [F, C], f32, kind="ExternalInput")
    b1r = nc.dram_tensor("b1r", [P, F // P], f32, kind="ExternalInput")
    b2r = nc.dram_tensor("b2r", [P, C], f32, kind="ExternalInput")
    wg = nc.dram_tensor("wg", [P, cap // P], f32, kind="ExternalInput")
    yg = nc.dram_tensor("yg", [cap, C], f32, kind="ExternalOutput")

    w1v = w1t.ap().rearrange("(co ci) f -> ci co f", ci=P)  # [128, 8, F]
    xgv = xgT.ap().rearrange("(co ci) n -> ci co n", ci=P)  # [128, 8, cap]

    with TileContext(nc) as tc:
        with (
            tc.tile_pool(name="consts", bufs=1) as consts,
            tc.tile_pool(name="wpool", bufs=4) as wpool,
            tc.tile_pool(name="xpool", bufs=2) as xpool,
            tc.tile_pool(name="hpool", bufs=3) as hpool,
            tc.tile_pool(name="ypool", bufs=3) as ypool,
            tc.tile_pool(name="psum_h", bufs=2, space="PSUM") as psum_h,
            tc.tile_pool(name="psum_y", bufs=1, space="PSUM") as psum_y,
        ):
            b1_sb = consts.tile([P, F // P], f32)
            nc.sync.dma_start(b1_sb[:], b1r[:, :])
            b2_sb = consts.tile([P, C], f32)
            nc.sync.dma_start(b2_sb[:], b2r[:, :])
            wg_sb = consts.tile([P, cap // P], f32)
            nc.sync.dma_start(wg_sb[:], wg[:, :])

            for s in range(nS):
                xg_s = xpool.tile([P, 8, SCH], f32r, tag="xg")
                nc.sync.dma_start(xg_s[:], xgv[:, :, s * SCH : (s + 1) * SCH].bitcast(f32r))

                yps = [
                    [
                        psum_y.tile(
                            [P, 512], f32, tag=f"y_{t}_{cc}", name=f"y_{t}_{cc}"
                        )
                        for cc in range(2)
                    ]
                    for t in range(3)
                ]

                for f in range(F // P):  # 32
                    w1c = wpool.tile([P, 8, P], f32r, tag="w1c")
                    nc.sync.dma_start(w1c[:], w1v[:, :, f * P : (f + 1) * P].bitcast(f32r))
                    w2c = wpool.tile([P, C], f32r, tag="w2c")
                    nc.sync.dma_start(w2c[:], w2t[f * P : (f + 1) * P, :].bitcast(f32r))

                    hps = psum_h.tile([P, SCH], f32, tag="h")
                    for c in range(8):
                        nc.tensor.matmul(
                            hps[:],
                            lhsT=w1c[:, c, :],
                            rhs=xg_s[:, c, :],
                            start=(c == 0),
                            stop=(c == 7),
                        )
                    hT = hpool.tile([P, SCH], f32r, tag="hT")
                    nc.scalar.activation(
                        hT[:],
                        hps[:],
                        mybir.ActivationFunctionType.Relu,
                        bias=b1_sb[:, f : f + 1],
                        scale=1.0,
                    )
                    for t in range(3):
                        for cc in range(2):
                            nc.tensor.matmul(
                                yps[t][cc][:],
                                lhsT=hT[:, t * P : (t + 1) * P],
                                rhs=w2c[:, cc * 512 : (cc + 1) * 512],
                                start=(f == 0),
                                stop=(f == F // P - 1),
                            )

                for t in range(3):
                    y_sb = ypool.tile([P, C], f32, tag="y_sb")
                    for cc in range(2):
                        sl = slice(cc * 512, (cc + 1) * 512)
                        nc.vector.tensor_add(y_sb[:, sl], yps[t][cc][:], b2_sb[:, sl])
                    yf = ypool.tile([P, C], f32, tag="yf")
                    nc.scalar.mul(yf[:], y_sb[:], wg_sb[:, s * 3 + t : s * 3 + t + 1])
                    nc.sync.dma_start(
                        yg[(s * 3 + t) * P : (s * 3 + t + 1) * P, :], yf[:]
                    )
    nc.compile()
    return nc




def _chunks(cap):
    sizes = [384] * (cap // 384)
    rem = cap - 384 * len(sizes)
    if rem:
        sizes.append(rem)  # runt chunk last: shortest possible drain tail
    return sizes


def _build_fast(cap: int):
    """Fast path (b1 == 0 and b2 == 0): inputs pre-gated and pre-tiled on host.

    All matmul operands are bf16 (quant error ~3e-3 << 2e-2 budget).  All
    weights stay resident in SBUF (128 KiB/partition), loaded once via
    per-fl tiles in consumption order; w1 issues on the gpsimd queue and w2
    on the vector queue so the sync queue (~600ns per dma_start) never
    serializes ahead of them.  Per-chunk y accumulates in PSUM across all
    32 f-tiles (6 banks y + 2 banks h), retired once per chunk.  A DVE
    memset feeds dummy matmuls at t~=0 so the PE HAM un-throttles before
    the first real matmul.
      inputs : xgf [cap*1024]  gated tokens bf16, per-chunk tiled [ci, co, n]
               w1p [32, 128, 8, 128]  w1.T tiled for mm1 lhsT (bf16)
               w2t [4096, 1024] (bf16)
      output : yg  [cap, 1024] f32
    """
    import concourse.mybir as mybir
    from concourse import bacc
    from concourse.tile import TileContext

    f32 = mybir.dt.float32
    bf16 = mybir.dt.bfloat16
    sizes = _chunks(cap)
    offs = [sum(sizes[:i]) for i in range(len(sizes))]
    NF = F // P  # 32
    NDUM = 24
    nc = bacc.Bacc(None, target_bir_lowering=False)

    xgf = nc.dram_tensor("xgf", [cap * C], bf16, kind="ExternalInput")
    w1p = nc.dram_tensor("w1p", [NF, P, 8, P], bf16, kind="ExternalInput")
    w2t = nc.dram_tensor("w2t", [F, C], bf16, kind="ExternalInput")
    yg = nc.dram_tensor("yg", [cap, C], bf16, kind="ExternalOutput")

    with TileContext(nc) as tc:
        with (
            tc.tile_pool(name="consts", bufs=1) as consts,
            tc.tile_pool(name="w1pool", bufs=NF) as w1pool,
            tc.tile_pool(name="w2pool", bufs=NF) as w2pool,
            tc.tile_pool(name="xhead", bufs=8) as xhead,
            tc.tile_pool(name="xpool", bufs=2) as xpool,
            tc.tile_pool(name="hpool", bufs=3) as hpool,
            tc.tile_pool(name="opool", bufs=3) as opool,
            tc.tile_pool(name="psum_h", bufs=2, space="PSUM") as psum_h,
            tc.tile_pool(name="psum_y", bufs=1, space="PSUM") as psum_y,
        ):
            warm = consts.tile([P, 256], bf16)
            nc.vector.memset(warm[:], 0.0)
            wps = psum_h.tile([P, 384], f32, tag="h", name="wps")

            def dummy_mms(n):
                # dependency-free matmuls: fill PE gaps while the head DMAs
                # trickle in so the HAM activity window never resets
                for _ in range(n):
                    nc.tensor.matmul(
                        wps[:, :P],
                        lhsT=warm[:, :P],
                        rhs=warm[:, :P],
                        start=True,
                        stop=True,
                    )

            dummy_mms(NDUM)

            def load_xg(s, split=False):
                sz = sizes[s]
                src = xgf[offs[s] * C : (offs[s] + sz) * C]
                v = src.rearrange("(ci co n) -> ci co n", ci=P, co=8)
                if split:  # head: per-co tiles so mm1(c) chases the DMA;
                    # spread issue over three queues (~650ns per descriptor
                    # serialization each); co2/co3 go on gpsimd AFTER w1f[0]
                    # (emitted by the caller interleave below)
                    ts = [
                        xhead.tile([P, sz], bf16, tag=f"xh{co}", name="xh")
                        for co in range(8)
                    ]
                    for co in (0, 1, 6, 7):
                        nc.sync.dma_start(ts[co][:], v[:, co, :])
                    for co in (4, 5):
                        nc.scalar.dma_start(ts[co][:], v[:, co, :])
                    pend = [(co, ts[co], v[:, co, :]) for co in (2, 3)]
                    return [t[:] for t in ts], pend
                xg_s = xpool.tile([P, 8, sz], bf16, tag="xg", name="xg_s")
                nc.sync.dma_start(xg_s[:], v)
                return [xg_s[:, co, :] for co in range(8)], []

            xg_cur, xpend = load_xg(0, split=True)
            w1f, w2f = [], []
            for fl in range(NF):
                t1 = w1pool.tile([P, 8, P], bf16, tag="w1f", name="w1f")
                nc.gpsimd.dma_start(t1[:], w1p[fl])
                w1f.append(t1)
                if fl == 0:  # co2/co3 right after w1f[0] on the gpsimd queue
                    for _, tile, view in xpend:
                        nc.gpsimd.dma_start(tile[:], view)
                t2 = w2pool.tile([P, C], bf16, tag="w2f", name="w2f")
                nc.sync.dma_start(t2[:], w2t[fl * P : (fl + 1) * P, :])
                w2f.append(t2)

            for s, sz in enumerate(sizes):
                nt = (sz + P - 1) // P
                xg_s = xg_cur
                if s + 1 < len(sizes):
                    xg_cur, _ = load_xg(s + 1)

                yps = [
                    psum_y.tile([P, C], f32, tag=f"y_{t}", name=f"y_{t}")
                    for t in range(nt)
                ]

                def mm2(fl, hT, yps=yps, nt=nt):
                    for t in range(nt):
                        for cc in range(2):
                            nc.tensor.matmul(
                                yps[t][:, cc * 512 : (cc + 1) * 512],
                                lhsT=hT[:, t * P : (t + 1) * P],
                                rhs=w2f[fl][:, cc * 512 : (cc + 1) * 512],
                                start=(fl == 0),
                                stop=(fl == NF - 1),
                            )

                # software pipeline: mm2 runs one fl behind mm1 (relu
                # latency covered by the next fl's mm1s)
                hT_prev = None
                for fl in range(NF):
                    hps = psum_h.tile([P, 384], f32, tag="h", name="hps")
                    if s == 0 and fl < 4:
                        # fill head DMA-pacing gaps so the HAM activity
                        # window never resets; the real mm1's start=True
                        # clears the region, so these are side-effect-free
                        for _ in range(8):
                            nc.tensor.matmul(
                                hps[:, :256],
                                lhsT=warm[:, :P],
                                rhs=warm[:],
                                start=True,
                                stop=True,
                            )
                    for c in range(8):
                        nc.tensor.matmul(
                            hps[:, :sz],
                            lhsT=w1f[fl][:, c, :],
                            rhs=xg_s[c],
                            start=(c == 0),
                            stop=(c == 7),
                        )
                    hT = hpool.tile([P, 384], bf16, tag="hT", name="hT")
                    if fl == NF - 1:
                        # last fl: per-token-tile relu so mm2(t) can
                        # start as soon as its slice is ready
                        for t in range(nt):
                            tl = slice(t * P, min((t + 1) * P, sz))
                            nc.scalar.activation(
                                hT[:, tl],
                                hps[:, tl],
                                mybir.ActivationFunctionType.Relu,
                            )
                    else:
                        nc.scalar.activation(
                            hT[:, :sz],
                            hps[:, :sz],
                            mybir.ActivationFunctionType.Relu,
                        )
                    if hT_prev is not None:
                        mm2(fl - 1, hT_prev)
                    hT_prev = hT
                mm2(NF - 1, hT_prev)
                last = s == len(sizes) - 1
                for t in range(nt):
                    yo = opool.tile([P, C], bf16, tag="yo", name="yo")
                    # last chunk: spread the PSUM->SBUF casts and the DMA
                    # issues over two engines so the drain doesn't serialize
                    if last and t % 2:
                        nc.scalar.copy(yo[:], yps[t][:])
                        w0 = offs[s] // P + t
                        nc.scalar.dma_start(yg[w0 * P : (w0 + 1) * P, :], yo[:])
                    else:
                        nc.vector.tensor_copy(yo[:], yps[t][:])
                        w0 = offs[s] // P + t
                        nc.sync.dma_start(yg[w0 * P : (w0 + 1) * P, :], yo[:])
    nc.compile()
    return nc


_CACHE = {}
_TRACE = False  # test harness sets True to capture an NTFF profile
_LAST_RES = None


def _get_nc(cap, fast):
    key = (cap, fast)
    if key not in _CACHE:
        _CACHE[key] = _build_fast(cap) if fast else _build(cap)
    return _CACHE[key]


def _route(x_flat, router_w):
    """Top-2 routing, float64 for stable selection. Returns idx/weights per expert."""
    logits = x_flat.astype(np.float64) @ router_w.astype(np.float64).T
    t = np.exp(logits - logits.max(-1, keepdims=True))
    p = t / t.sum(-1, keepdims=True)
    top2 = np.argsort(-p, axis=-1)[:, :2]
    pv = np.take_along_axis(p, top2, axis=-1)
    wn = pv / (pv.sum(-1, keepdims=True) + 1e-9)
    return top2, wn


def kernel(x, router_w, w1, b1, w2, b2):
    from concourse.bass_utils import run_bass_kernel_spmd

    Bx, Nx, Cx = x.shape
    x_flat = np.ascontiguousarray(x.reshape(-1, Cx))
    T = x_flat.shape[0]

    top2, wn = _route(x_flat, router_w)
    idxs, gates = [], []
    for e in range(E):
        sel = top2 == e
        we = np.where(sel, wn, 0.0).sum(-1)
        idx = np.nonzero(sel.any(-1))[0]
        idxs.append(idx)
        gates.append(we[idx].astype(np.float32))
    cap = max(len(i) for i in idxs)
    fastcap = ((cap + P - 1) // P) * P
    cap = ((cap + SCH - 1) // SCH) * SCH

    fast = bool(np.all(b1 == 0) and np.all(b2 == 0))
    if fast:
        cap = fastcap
    nc = _get_nc(cap, fast)

    in_maps = []
    for e in range(E):
        n_e = len(idxs[e])
        xg = np.zeros((cap, Cx), np.float32)
        xg[:n_e] = x_flat[idxs[e]]
        wg = np.zeros(cap, np.float32)
        wg[:n_e] = gates[e]
        if fast:
            import ml_dtypes

            bf16 = ml_dtypes.bfloat16
            xg *= wg[:, None]  # pre-gate: exact since b1 == 0 and wg >= 0
            sizes = _chunks(cap)
            blocks, off = [], 0
            for sz in sizes:
                blocks.append(
                    np.ascontiguousarray(
                        xg[off : off + sz].reshape(sz, 8, P).transpose(2, 1, 0)
                    )
                    .ravel()
                    .astype(bf16)
                )
                off += sz
            in_maps.append(
                {
                    "xgf": np.concatenate(blocks),
                    "w1p": np.ascontiguousarray(
                        w1[e].reshape(F // P, P, 8, P).transpose(0, 3, 2, 1)
                    ).astype(bf16),
                    "w2t": np.ascontiguousarray(w2[e].T).astype(bf16),
                }
            )
        else:
            in_maps.append(
                {
                    "xgT": np.ascontiguousarray(xg.T),
                    "w1t": np.ascontiguousarray(w1[e].T),
                    "w2t": np.ascontiguousarray(w2[e].T),
                    "b1r": np.ascontiguousarray(b1[e].reshape(F // P, P).T),
                    "b2r": np.ascontiguousarray(np.broadcast_to(b2[e], (P, Cx))),
                    "wg": np.ascontiguousarray(wg.reshape(cap // P, P).T),
                }
            )

    global _LAST_RES
    res = run_bass_kernel_spmd(nc, in_maps, core_ids=list(range(E)), trace=_TRACE)
    _LAST_RES = res

    out = np.zeros((T, Cx), np.float32)
    for e in range(E):
        n_e = len(idxs[e])
        out[idxs[e]] += res.results[e]["yg"][:n_e].astype(np.float32)
    return out.reshape(Bx, Nx, Cx)



# revision 31
# speedup vs baseline: 1.0092x; 1.0003x over previous
"""MoE layer (B=4, N=2048, C=1024, F=4096, E=8, top-2) on 8 trn2 NeuronCores.

Sharding: expert-parallel. The host computes the (tiny, ~0.1% of FLOPs)
router and dispatches each expert's tokens to one core as part of sharding;
each core runs its expert's full FFN  relu(xg @ w1.T + b1) @ w2.T + b2,
gated by the combine weight, over its gathered tokens.  The host combine
scatter-adds the per-expert results back into the full output.

Device kernel (per core, SPMD, identical program):
  inputs : xgT [1024, cap]  (gathered tokens, transposed)
           w1t [1024, 4096] (w1[e].T)   w2t [4096, 1024] (w2[e].T)
           b1r [128, 32]    (b1[e] partition-major)
           b2r [128, 1024]  (b2[e] replicated over partitions)
           wg  [128, cap/128] (combine weights, partition-major)
  output : yg  [cap, 1024]
Matmuls run as float32r (FP22 multiply, fp32 accumulate) — 4x the fp32 rate.
"""

import numpy as np

P = 128
C = 1024
F = 4096
E = 8
SCH = 384  # token chunk: 3 PSUM banks (x 2 C-halves) for y + 2 for h = 8


def _build(cap: int):
    import concourse.mybir as mybir
    from concourse import bacc
    from concourse.tile import TileContext

    f32 = mybir.dt.float32
    f32r = mybir.dt.float32r
    nS = cap // SCH
    nc = bacc.Bacc(None, target_bir_lowering=False)

    xgT = nc.dram_tensor("xgT", [C, cap], f32, kind="ExternalInput")
    w1t = nc.dram_tensor("w1t", [C, F], f32, kind="ExternalInput")
    w2t = nc.dram_tensor("w2t", [F, C], f32, kind="ExternalInput")
    b1r = nc.dram_tensor("b1r", [P, F // P], f32, kind="ExternalInput")
    b2r = nc.dram_tensor("b2r", [P, C], f32, kind="ExternalInput")
    wg = nc.dram_tensor("wg", [P, cap // P], f32, kind="ExternalInput")
    yg = nc.dram_tensor("yg", [cap, C], f32, kind="ExternalOutput")

    w1v = w1t.ap().rearrange("(co ci) f -> ci co f", ci=P)  # [128, 8, F]
    xgv = xgT.ap().rearrange("(co ci) n -> ci co n", ci=P)  # [128, 8, cap]

    with TileContext(nc) as tc:
        with (
            tc.tile_pool(name="consts", bufs=1) as consts,
            tc.tile_pool(name="wpool", bufs=4) as wpool,
            tc.tile_pool(name="xpool", bufs=2) as xpool,
            tc.tile_pool(name="hpool", bufs=3) as hpool,
            tc.tile_pool(name="ypool", bufs=3) as ypool,
            tc.tile_pool(name="psum_h", bufs=2, space="PSUM") as psum_h,
            tc.tile_pool(name="psum_y", bufs=1, space="PSUM") as psum_y,
        ):
            b1_sb = consts.tile([P, F // P], f32)
            nc.sync.dma_start(b1_sb[:], b1r[:, :])
            b2_sb = consts.tile([P, C], f32)
            nc.sync.dma_start(b2_sb[:], b2r[:, :])
            wg_sb = consts.tile([P, cap // P], f32)
            nc.sync.dma_start(wg_sb[:], wg[:, :])

            for s in range(nS):
                xg_s = xpool.tile([P, 8, SCH], f32r, tag="xg")
                nc.sync.dma_start(xg_s[:], xgv[:, :, s * SCH : (s + 1) * SCH].bitcast(f32r))

                yps = [
                    [
                        psum_y.tile(
                            [P, 512], f32, tag=f"y_{t}_{cc}", name=f"y_{t}_{cc}"
                        )
                        for cc in range(2)
                    ]
                    for t in range(3)
                ]

                for f in range(F // P):  # 32
                    w1c = wpool.tile([P, 8, P], f32r, tag="w1c")
                    nc.sync.dma_start(w1c[:], w1v[:, :, f * P : (f + 1) * P].bitcast(f32r))
                    w2c = wpool.tile([P, C], f32r, tag="w2c")
                    nc.sync.dma_start(w2c[:], w2t[f * P : (f + 1) * P, :].bitcast(f32r))

                    hps = psum_h.tile([P, SCH], f32, tag="h")
                    for c in range(8):
                        nc.tensor.matmul(
                            hps[:],
                            lhsT=w1c[:, c, :],
                            rhs=xg_s[:, c, :],
                            start=(c == 0),
                            stop=(c == 7),
                        )
                    hT = hpool.tile([P, SCH], f32r, tag="hT")
                    nc.scalar.activation(
                        hT[:],
                        hps[:],
                        mybir.ActivationFunctionType.Relu,
                        bias=b1_sb[:, f : f + 1],
                        scale=1.0,
                    )
                    for t in range(3):
                        for cc in range(2):
                            nc.tensor.matmul(
                                yps[t][cc][:],
                                lhsT=hT[:, t * P : (t + 1) * P],
                                rhs=w2c[:, cc * 512 : (cc + 1) * 512],
                                start=(f == 0),
                                stop=(f == F // P - 1),
                            )

                for t in range(3):
                    y_sb = ypool.tile([P, C], f32, tag="y_sb")
                    for cc in range(2):
                        sl = slice(cc * 512, (cc + 1) * 512)
                        nc.vector.tensor_add(y_sb[:, sl], yps[t][cc][:], b2_sb[:, sl])
                    yf = ypool.tile([P, C], f32, tag="yf")
                    nc.scalar.mul(yf[:], y_sb[:], wg_sb[:, s * 3 + t : s * 3 + t + 1])
                    nc.sync.dma_start(
                        yg[(s * 3 + t) * P : (s * 3 + t + 1) * P, :], yf[:]
                    )
    nc.compile()
    return nc




def _chunks(cap):
    sizes = [384] * (cap // 384)
    rem = cap - 384 * len(sizes)
    if rem:
        sizes.append(rem)  # runt chunk last: shortest possible drain tail
    return sizes


def _build_fast(cap: int):
    """Fast path (b1 == 0 and b2 == 0): inputs pre-gated and pre-tiled on host.

    All matmul operands are bf16 (quant error ~3e-3 << 2e-2 budget).  All
    weights stay resident in SBUF (128 KiB/partition), loaded once via
    per-fl tiles in consumption order; w1 issues on the gpsimd queue and w2
    on the vector queue so the sync queue (~600ns per dma_start) never
    serializes ahead of them.  Per-chunk y accumulates in PSUM across all
    32 f-tiles (6 banks y + 2 banks h), retired once per chunk.  A DVE
    memset feeds dummy matmuls at t~=0 so the PE HAM un-throttles before
    the first real matmul.
      inputs : xgf [cap*1024]  gated tokens bf16, per-chunk tiled [ci, co, n]
               w1p [32, 128, 8, 128]  w1.T tiled for mm1 lhsT (bf16)
               w2t [4096, 1024] (bf16)
      output : yg  [cap, 1024] f32
    """
    import concourse.mybir as mybir
    from concourse import bacc
    from concourse.tile import TileContext

    f32 = mybir.dt.float32
    bf16 = mybir.dt.bfloat16
    sizes = _chunks(cap)
    offs = [sum(sizes[:i]) for i in range(len(sizes))]
    NF = F // P  # 32
    NDUM = 24
    nc = bacc.Bacc(None, target_bir_lowering=False)

    xgf = nc.dram_tensor("xgf", [cap * C], bf16, kind="ExternalInput")
    w1p = nc.dram_tensor("w1p", [NF, P, 8, P], bf16, kind="ExternalInput")
    w2t = nc.dram_tensor("w2t", [F, C], bf16, kind="ExternalInput")
    yg = nc.dram_tensor("yg", [cap, C], bf16, kind="ExternalOutput")

    with TileContext(nc) as tc:
        with (
            tc.tile_pool(name="consts", bufs=1) as consts,
            tc.tile_pool(name="w1pool", bufs=NF) as w1pool,
            tc.tile_pool(name="w2pool", bufs=NF) as w2pool,
            tc.tile_pool(name="xhead", bufs=8) as xhead,
            tc.tile_pool(name="xpool", bufs=2) as xpool,
            tc.tile_pool(name="hpool", bufs=3) as hpool,
            tc.tile_pool(name="opool", bufs=3) as opool,
            tc.tile_pool(name="psum_h", bufs=2, space="PSUM") as psum_h,
            tc.tile_pool(name="psum_y", bufs=1, space="PSUM") as psum_y,
        ):
            warm = consts.tile([P, 256], bf16)
            nc.vector.memset(warm[:], 0.0)
            wps = psum_h.tile([P, 384], f32, tag="h", name="wps")

            def dummy_mms(n):
                # dependency-free matmuls: fill PE gaps while the head DMAs
                # trickle in so the HAM activity window never resets
                for _ in range(n):
                    nc.tensor.matmul(
                        wps[:, :P],
                        lhsT=warm[:, :P],
                        rhs=warm[:, :P],
                        start=True,
                        stop=True,
                    )

            dummy_mms(NDUM)

            def load_xg(s, split=False):
                sz = sizes[s]
                src = xgf[offs[s] * C : (offs[s] + sz) * C]
                v = src.rearrange("(ci co n) -> ci co n", ci=P, co=8)
                if split:  # head: per-co tiles so mm1(c) chases the DMA;
                    # spread issue over three queues (~650ns per descriptor
                    # serialization each); co2/co3 go on gpsimd AFTER w1f[0]
                    # (emitted by the caller interleave below)
                    ts = [
                        xhead.tile([P, sz], bf16, tag=f"xh{co}", name="xh")
                        for co in range(8)
                    ]
                    for co in (0, 1, 6, 7):
                        nc.sync.dma_start(ts[co][:], v[:, co, :])
                    for co in (4, 5):
                        nc.scalar.dma_start(ts[co][:], v[:, co, :])
                    pend = [(co, ts[co], v[:, co, :]) for co in (2, 3)]
                    return [t[:] for t in ts], pend
                xg_s = xpool.tile([P, 8, sz], bf16, tag="xg", name="xg_s")
                nc.sync.dma_start(xg_s[:], v)
                return [xg_s[:, co, :] for co in range(8)], []

            xg_cur, xpend = load_xg(0, split=True)
            w1f, w2f = [], []
            for fl in range(NF):
                t1 = w1pool.tile([P, 8, P], bf16, tag="w1f", name="w1f")
                nc.gpsimd.dma_start(t1[:], w1p[fl])
                w1f.append(t1)
                if fl == 0:  # co2/co3 right after w1f[0] on the gpsimd queue
                    for _, tile, view in xpend:
                        nc.gpsimd.dma_start(tile[:], view)
                t2 = w2pool.tile([P, C], bf16, tag="w2f", name="w2f")
                nc.sync.dma_start(t2[:], w2t[fl * P : (fl + 1) * P, :])
                w2f.append(t2)

            for s, sz in enumerate(sizes):
                nt = (sz + P - 1) // P
                xg_s = xg_cur
                if s + 1 < len(sizes):
                    xg_cur, _ = load_xg(s + 1)

                yps = [
                    psum_y.tile([P, C], f32, tag=f"y_{t}", name=f"y_{t}")
                    for t in range(nt)
                ]

                def mm2(fl, hT, yps=yps, nt=nt):
                    for t in range(nt):
                        for cc in range(2):
                            nc.tensor.matmul(
                                yps[t][:, cc * 512 : (cc + 1) * 512],
                                lhsT=hT[:, t * P : (t + 1) * P],
                                rhs=w2f[fl][:, cc * 512 : (cc + 1) * 512],
                                start=(fl == 0),
                                stop=(fl == NF - 1),
                            )

                # software pipeline: mm2 runs one fl behind mm1 (relu
                # latency covered by the next fl's mm1s)
                hT_prev = None
                for fl in range(NF):
                    hps = psum_h.tile([P, 384], f32, tag="h", name="hps")
                    if s == 0 and fl < 4:
                        # fill head DMA-pacing gaps so the HAM activity
                        # window never resets; the real mm1's start=True
                        # clears the region, so these are side-effect-free
                        for _ in range(4):
                            nc.tensor.matmul(
                                hps[:, :256],
                                lhsT=warm[:, :P],
                                rhs=warm[:],
                                start=True,
                                stop=True,
                            )
                    for c in range(8):
                        nc.tensor.matmul(
                            hps[:, :sz],
                            lhsT=w1f[fl][:, c, :],
                            rhs=xg_s[c],
                            start=(c == 0),
                            stop=(c == 7),
                        )
                    hT = hpool.tile([P, 384], bf16, tag="hT", name="hT")
                    if fl == NF - 1:
                        # last fl: per-token-tile relu so mm2(t) can
                        # start as soon as its slice is ready
                        for t in range(nt):
                            tl = slice(t * P, min((t + 1) * P, sz))
                            nc.scalar.activation(
                                hT[:, tl],
                                hps[:, tl],
                                mybir.ActivationFunctionType.Relu,
                            )
                    else:
                        nc.scalar.activation(
                            hT[:, :sz],
                            hps[:, :sz],
                            mybir.ActivationFunctionType.Relu,
                        )
                    if hT_prev is not None:
                        mm2(fl - 1, hT_prev)
                    hT_prev = hT
                mm2(NF - 1, hT_prev)
                last = s == len(sizes) - 1
                for t in range(nt):
                    yo = opool.tile([P, C], bf16, tag="yo", name="yo")
                    # last chunk: spread the PSUM->SBUF casts and the DMA
                    # issues over two engines so the drain doesn't serialize
                    if last and t % 2:
                        nc.scalar.copy(yo[:], yps[t][:])
                        w0 = offs[s] // P + t
                        nc.scalar.dma_start(yg[w0 * P : (w0 + 1) * P, :], yo[:])
                    else:
                        nc.vector.tensor_copy(yo[:], yps[t][:])
                        w0 = offs[s] // P + t
                        nc.sync.dma_start(yg[w0 * P : (w0 + 1) * P, :], yo[:])
    nc.compile()
    return nc


_CACHE = {}
_TRACE = False  # test harness sets True to capture an NTFF profile
_LAST_RES = None


def _get_nc(cap, fast):
    key = (cap, fast)
    if key not in _CACHE:
        _CACHE[key] = _build_fast(cap) if fast else _build(cap)
    return _CACHE[key]


def _route(x_flat, router_w):
    """Top-2 routing, float64 for stable selection. Returns idx/weights per expert."""
    logits = x_flat.astype(np.float64) @ router_w.astype(np.float64).T
    t = np.exp(logits - logits.max(-1, keepdims=True))
    p = t / t.sum(-1, keepdims=True)
    top2 = np.argsort(-p, axis=-1)[:, :2]
    pv = np.take_along_axis(p, top2, axis=-1)
    wn = pv / (pv.sum(-1, keepdims=True) + 1e-9)
    return top2, wn


def kernel(x, router_w, w1, b1, w2, b2):
    from concourse.bass_utils import run_bass_kernel_spmd

    Bx, Nx, Cx = x.shape
    x_flat = np.ascontiguousarray(x.reshape(-1, Cx))
    T = x_flat.shape[0]

    top2, wn = _route(x_flat, router_w)
    idxs, gates = [], []
    for e in range(E):
        sel = top2 == e
        we = np.where(sel, wn, 0.0).sum(-1)
        idx = np.nonzero(sel.any(-1))[0]
        idxs.append(idx)
        gates.append(we[idx].astype(np.float32))
    cap = max(len(i) for i in idxs)
    fastcap = ((cap + P - 1) // P) * P
    cap = ((cap + SCH - 1) // SCH) * SCH

    fast = bool(np.all(b1 == 0) and np.all(b2 == 0))
    if fast:
        cap = fastcap
    nc = _get_nc(cap, fast)

    in_maps = []
    for e in range(E):
        n_e = len(idxs[e])
        xg = np.zeros((cap, Cx), np.float32)
        xg[:n_e] = x_flat[idxs[e]]
        wg = np.zeros(cap, np.float32)
        wg[:n_e] = gates[e]
        if fast:
            import ml_dtypes

            bf16 = ml_dtypes.bfloat16
            xg *= wg[:, None]  # pre-gate: exact since b1 == 0 and wg >= 0
            sizes = _chunks(cap)
            blocks, off = [], 0
            for sz in sizes:
                blocks.append(
                    np.ascontiguousarray(
                        xg[off : off + sz].reshape(sz, 8, P).transpose(2, 1, 0)
                    )
                    .ravel()
                    .astype(bf16)
                )
                off += sz
            in_maps.append(
                {
                    "xgf": np.concatenate(blocks),
                    "w1p": np.ascontiguousarray(
                        w1[e].reshape(F // P, P, 8, P).transpose(0, 3, 2, 1)
                    ).astype(bf16),
                    "w2t": np.ascontiguousarray(w2[e].T).astype(bf16),
                }
            )
        else:
            in_maps.append(
                {
                    "xgT": np.ascontiguousarray(xg.T),
                    "w1t": np.ascontiguousarray(w1[e].T),
                    "w2t": np.ascontiguousarray(w2[e].T),
                    "b1r": np.ascontiguousarray(b1[e].reshape(F // P, P).T),
                    "b2r": np.ascontiguousarray(np.broadcast_to(b2[e], (P, Cx))),
                    "wg": np.ascontiguousarray(wg.reshape(cap // P, P).T),
                }
            )

    global _LAST_RES
    res = run_bass_kernel_spmd(nc, in_maps, core_ids=list(range(E)), trace=_TRACE)
    _LAST_RES = res

    out = np.zeros((T, Cx), np.float32)
    for e in range(E):
        n_e = len(idxs[e])
        out[idxs[e]] += res.results[e]["yg"][:n_e].astype(np.float32)
    return out.reshape(Bx, Nx, Cx)



# revision 32
# speedup vs baseline: 1.0101x; 1.0009x over previous
"""MoE layer (B=4, N=2048, C=1024, F=4096, E=8, top-2) on 8 trn2 NeuronCores.

Sharding: expert-parallel. The host computes the (tiny, ~0.1% of FLOPs)
router and dispatches each expert's tokens to one core as part of sharding;
each core runs its expert's full FFN  relu(xg @ w1.T) @ w2.T over its
gathered, pre-gated tokens.  The host combine scatter-adds the per-expert
results back into the full output.

Fast path (b1 == b2 == 0, the spec'd fill): all matmul operands bf16
(error ~4e-3 << the 2e-2 budget, same 78.6 TF/s PE rate as f32r).  The
kernel is PE-streaming-bound: per core ~2176 tokens x 512 col-pumps =
464us of matmul at 2.4 GHz, and everything else is engineered to hide
behind that stream:
  - all expert weights stay resident in SBUF (128 KiB/partition), loaded
    once via per-f-tile DMAs in consumption order, issued across three
    engine queues (sync/scalar/gpsimd, ~650ns per descriptor each);
  - per 384-token chunk, y accumulates in PSUM across all 32 f-tiles
    (6 banks y + 2 banks h = all 8), retired once per chunk to bf16;
  - dependency-free warmup matmuls on a DVE-memset zero tile keep the PE
    HAM activity window unbroken until the head DMAs land (else the PE
    clock-gates to 1.2 GHz);
  - the tail chunk's PSUM casts + output DMAs are spread over DVE+ACT.
The slow path (nonzero biases) keeps the original f32r streaming kernel.
"""

import numpy as np

P = 128
C = 1024
F = 4096
E = 8
SCH = 384  # token chunk: 3 PSUM banks (x 2 C-halves) for y + 2 for h = 8


def _build(cap: int):
    import concourse.mybir as mybir
    from concourse import bacc
    from concourse.tile import TileContext

    f32 = mybir.dt.float32
    f32r = mybir.dt.float32r
    nS = cap // SCH
    nc = bacc.Bacc(None, target_bir_lowering=False)

    xgT = nc.dram_tensor("xgT", [C, cap], f32, kind="ExternalInput")
    w1t = nc.dram_tensor("w1t", [C, F], f32, kind="ExternalInput")
    w2t = nc.dram_tensor("w2t", [F, C], f32, kind="ExternalInput")
    b1r = nc.dram_tensor("b1r", [P, F // P], f32, kind="ExternalInput")
    b2r = nc.dram_tensor("b2r", [P, C], f32, kind="ExternalInput")
    wg = nc.dram_tensor("wg", [P, cap // P], f32, kind="ExternalInput")
    yg = nc.dram_tensor("yg", [cap, C], f32, kind="ExternalOutput")

    w1v = w1t.ap().rearrange("(co ci) f -> ci co f", ci=P)  # [128, 8, F]
    xgv = xgT.ap().rearrange("(co ci) n -> ci co n", ci=P)  # [128, 8, cap]

    with TileContext(nc) as tc:
        with (
            tc.tile_pool(name="consts", bufs=1) as consts,
            tc.tile_pool(name="wpool", bufs=4) as wpool,
            tc.tile_pool(name="xpool", bufs=2) as xpool,
            tc.tile_pool(name="hpool", bufs=3) as hpool,
            tc.tile_pool(name="ypool", bufs=3) as ypool,
            tc.tile_pool(name="psum_h", bufs=2, space="PSUM") as psum_h,
            tc.tile_pool(name="psum_y", bufs=1, space="PSUM") as psum_y,
        ):
            b1_sb = consts.tile([P, F // P], f32)
            nc.sync.dma_start(b1_sb[:], b1r[:, :])
            b2_sb = consts.tile([P, C], f32)
            nc.sync.dma_start(b2_sb[:], b2r[:, :])
            wg_sb = consts.tile([P, cap // P], f32)
            nc.sync.dma_start(wg_sb[:], wg[:, :])

            for s in range(nS):
                xg_s = xpool.tile([P, 8, SCH], f32r, tag="xg")
                nc.sync.dma_start(xg_s[:], xgv[:, :, s * SCH : (s + 1) * SCH].bitcast(f32r))

                yps = [
                    [
                        psum_y.tile(
                            [P, 512], f32, tag=f"y_{t}_{cc}", name=f"y_{t}_{cc}"
                        )
                        for cc in range(2)
                    ]
                    for t in range(3)
                ]

                for f in range(F // P):  # 32
                    w1c = wpool.tile([P, 8, P], f32r, tag="w1c")
                    nc.sync.dma_start(w1c[:], w1v[:, :, f * P : (f + 1) * P].bitcast(f32r))
                    w2c = wpool.tile([P, C], f32r, tag="w2c")
                    nc.sync.dma_start(w2c[:], w2t[f * P : (f + 1) * P, :].bitcast(f32r))

                    hps = psum_h.tile([P, SCH], f32, tag="h")
                    for c in range(8):
                        nc.tensor.matmul(
                            hps[:],
                            lhsT=w1c[:, c, :],
                            rhs=xg_s[:, c, :],
                            start=(c == 0),
                            stop=(c == 7),
                        )
                    hT = hpool.tile([P, SCH], f32r, tag="hT")
                    nc.scalar.activation(
                        hT[:],
                        hps[:],
                        mybir.ActivationFunctionType.Relu,
                        bias=b1_sb[:, f : f + 1],
                        scale=1.0,
                    )
                    for t in range(3):
                        for cc in range(2):
                            nc.tensor.matmul(
                                yps[t][cc][:],
                                lhsT=hT[:, t * P : (t + 1) * P],
                                rhs=w2c[:, cc * 512 : (cc + 1) * 512],
                                start=(f == 0),
                                stop=(f == F // P - 1),
                            )

                for t in range(3):
                    y_sb = ypool.tile([P, C], f32, tag="y_sb")
                    for cc in range(2):
                        sl = slice(cc * 512, (cc + 1) * 512)
                        nc.vector.tensor_add(y_sb[:, sl], yps[t][cc][:], b2_sb[:, sl])
                    yf = ypool.tile([P, C], f32, tag="yf")
                    nc.scalar.mul(yf[:], y_sb[:], wg_sb[:, s * 3 + t : s * 3 + t + 1])
                    nc.sync.dma_start(
                        yg[(s * 3 + t) * P : (s * 3 + t + 1) * P, :], yf[:]
                    )
    nc.compile()
    return nc




def _chunks(cap):
    sizes = [384] * (cap // 384)
    rem = cap - 384 * len(sizes)
    if rem:
        sizes.append(rem)  # runt chunk last: shortest possible drain tail
    return sizes


def _build_fast(cap: int):
    """Fast path (b1 == 0 and b2 == 0): inputs pre-gated and pre-tiled on host.

    All matmul operands are bf16 (quant error ~3e-3 << 2e-2 budget).  All
    weights stay resident in SBUF (128 KiB/partition), loaded once via
    per-fl tiles in consumption order; w1 issues on the gpsimd queue and w2
    on the vector queue so the sync queue (~600ns per dma_start) never
    serializes ahead of them.  Per-chunk y accumulates in PSUM across all
    32 f-tiles (6 banks y + 2 banks h), retired once per chunk.  A DVE
    memset feeds dummy matmuls at t~=0 so the PE HAM un-throttles before
    the first real matmul.
      inputs : xgf [cap*1024]  gated tokens bf16, per-chunk tiled [ci, co, n]
               w1p [32, 128, 8, 128]  w1.T tiled for mm1 lhsT (bf16)
               w2t [4096, 1024] (bf16)
      output : yg  [cap, 1024] f32
    """
    import concourse.mybir as mybir
    from concourse import bacc
    from concourse.tile import TileContext

    f32 = mybir.dt.float32
    bf16 = mybir.dt.bfloat16
    sizes = _chunks(cap)
    offs = [sum(sizes[:i]) for i in range(len(sizes))]
    NF = F // P  # 32
    NDUM = 24
    nc = bacc.Bacc(None, target_bir_lowering=False)

    xgf = nc.dram_tensor("xgf", [cap * C], bf16, kind="ExternalInput")
    w1p = nc.dram_tensor("w1p", [NF, P, 8, P], bf16, kind="ExternalInput")
    w2t = nc.dram_tensor("w2t", [F, C], bf16, kind="ExternalInput")
    yg = nc.dram_tensor("yg", [cap, C], bf16, kind="ExternalOutput")

    with TileContext(nc) as tc:
        with (
            tc.tile_pool(name="consts", bufs=1) as consts,
            tc.tile_pool(name="w1pool", bufs=NF) as w1pool,
            tc.tile_pool(name="w2pool", bufs=NF) as w2pool,
            tc.tile_pool(name="xhead", bufs=8) as xhead,
            tc.tile_pool(name="xpool", bufs=2) as xpool,
            tc.tile_pool(name="hpool", bufs=3) as hpool,
            tc.tile_pool(name="opool", bufs=3) as opool,
            tc.tile_pool(name="psum_h", bufs=2, space="PSUM") as psum_h,
            tc.tile_pool(name="psum_y", bufs=1, space="PSUM") as psum_y,
        ):
            warm = consts.tile([P, 256], bf16)
            nc.vector.memset(warm[:], 0.0)
            wps = psum_h.tile([P, 384], f32, tag="h", name="wps")

            def dummy_mms(n):
                # dependency-free matmuls: fill PE gaps while the head DMAs
                # trickle in so the HAM activity window never resets
                for _ in range(n):
                    nc.tensor.matmul(
                        wps[:, :P],
                        lhsT=warm[:, :P],
                        rhs=warm[:, :P],
                        start=True,
                        stop=True,
                    )

            dummy_mms(NDUM)

            def load_xg(s, split=False):
                sz = sizes[s]
                src = xgf[offs[s] * C : (offs[s] + sz) * C]
                v = src.rearrange("(ci co n) -> ci co n", ci=P, co=8)
                if split:  # head: per-co tiles so mm1(c) chases the DMA;
                    # spread issue over three queues (~650ns per descriptor
                    # serialization each); co2/co3 go on gpsimd AFTER w1f[0]
                    # (emitted by the caller interleave below)
                    ts = [
                        xhead.tile([P, sz], bf16, tag=f"xh{co}", name="xh")
                        for co in range(8)
                    ]
                    for co in (0, 1, 6, 7):
                        nc.sync.dma_start(ts[co][:], v[:, co, :])
                    for co in (4, 5):
                        nc.scalar.dma_start(ts[co][:], v[:, co, :])
                    pend = [(co, ts[co], v[:, co, :]) for co in (2, 3)]
                    return [t[:] for t in ts], pend
                xg_s = xpool.tile([P, 8, sz], bf16, tag="xg", name="xg_s")
                nc.sync.dma_start(xg_s[:], v)
                return [xg_s[:, co, :] for co in range(8)], []

            xg_cur, xpend = load_xg(0, split=True)
            w1f, w2f = [], []
            for fl in range(NF):
                t1 = w1pool.tile([P, 8, P], bf16, tag="w1f", name="w1f")
                nc.gpsimd.dma_start(t1[:], w1p[fl])
                w1f.append(t1)
                if fl == 0:  # co2/co3 right after w1f[0] on the gpsimd queue
                    for _, tile, view in xpend:
                        nc.gpsimd.dma_start(tile[:], view)
                t2 = w2pool.tile([P, C], bf16, tag="w2f", name="w2f")
                nc.sync.dma_start(t2[:], w2t[fl * P : (fl + 1) * P, :])
                w2f.append(t2)

            for s, sz in enumerate(sizes):
                nt = (sz + P - 1) // P
                xg_s = xg_cur
                if s + 1 < len(sizes):
                    xg_cur, _ = load_xg(s + 1)

                yps = [
                    psum_y.tile([P, C], f32, tag=f"y_{t}", name=f"y_{t}")
                    for t in range(nt)
                ]

                def mm2(fl, hT, yps=yps, nt=nt):
                    for t in range(nt):
                        for cc in range(2):
                            nc.tensor.matmul(
                                yps[t][:, cc * 512 : (cc + 1) * 512],
                                lhsT=hT[:, t * P : (t + 1) * P],
                                rhs=w2f[fl][:, cc * 512 : (cc + 1) * 512],
                                start=(fl == 0),
                                stop=(fl == NF - 1),
                            )

                # software pipeline: mm2 runs one fl behind mm1 (relu
                # latency covered by the next fl's mm1s)
                hT_prev = None
                for fl in range(NF):
                    hps = psum_h.tile([P, 384], f32, tag="h", name="hps")
                    if s == 0 and fl < 4:
                        # fill head DMA-pacing gaps so the HAM activity
                        # window never resets; the real mm1's start=True
                        # clears the region, so these are side-effect-free
                        for _ in range(4):
                            nc.tensor.matmul(
                                hps[:, :256],
                                lhsT=warm[:, :P],
                                rhs=warm[:],
                                start=True,
                                stop=True,
                            )
                    for c in range(8):
                        nc.tensor.matmul(
                            hps[:, :sz],
                            lhsT=w1f[fl][:, c, :],
                            rhs=xg_s[c],
                            start=(c == 0),
                            stop=(c == 7),
                        )
                    hT = hpool.tile([P, 384], bf16, tag="hT", name="hT")
                    if fl == NF - 1:
                        # last fl: per-token-tile relu so mm2(t) can
                        # start as soon as its slice is ready
                        for t in range(nt):
                            tl = slice(t * P, min((t + 1) * P, sz))
                            nc.scalar.activation(
                                hT[:, tl],
                                hps[:, tl],
                                mybir.ActivationFunctionType.Relu,
                            )
                    else:
                        nc.scalar.activation(
                            hT[:, :sz],
                            hps[:, :sz],
                            mybir.ActivationFunctionType.Relu,
                        )
                    if hT_prev is not None:
                        mm2(fl - 1, hT_prev)
                    hT_prev = hT
                mm2(NF - 1, hT_prev)
                last = s == len(sizes) - 1
                for t in range(nt):
                    yo = opool.tile([P, C], bf16, tag="yo", name="yo")
                    # last chunk: spread the PSUM->SBUF casts and the DMA
                    # issues over two engines so the drain doesn't serialize
                    if last and t % 2:
                        nc.scalar.copy(yo[:], yps[t][:])
                        w0 = offs[s] // P + t
                        nc.scalar.dma_start(yg[w0 * P : (w0 + 1) * P, :], yo[:])
                    else:
                        nc.vector.tensor_copy(yo[:], yps[t][:])
                        w0 = offs[s] // P + t
                        nc.sync.dma_start(yg[w0 * P : (w0 + 1) * P, :], yo[:])
    nc.compile()
    return nc


_CACHE = {}
_TRACE = False  # test harness sets True to capture an NTFF profile
_LAST_RES = None


def _get_nc(cap, fast):
    key = (cap, fast)
    if key not in _CACHE:
        _CACHE[key] = _build_fast(cap) if fast else _build(cap)
    return _CACHE[key]


def _route(x_flat, router_w):
    """Top-2 routing, float64 for stable selection. Returns idx/weights per expert."""
    logits = x_flat.astype(np.float64) @ router_w.astype(np.float64).T
    t = np.exp(logits - logits.max(-1, keepdims=True))
    p = t / t.sum(-1, keepdims=True)
    top2 = np.argsort(-p, axis=-1)[:, :2]
    pv = np.take_along_axis(p, top2, axis=-1)
    wn = pv / (pv.sum(-1, keepdims=True) + 1e-9)
    return top2, wn


def kernel(x, router_w, w1, b1, w2, b2):
    from concourse.bass_utils import run_bass_kernel_spmd

    Bx, Nx, Cx = x.shape
    x_flat = np.ascontiguousarray(x.reshape(-1, Cx))
    T = x_flat.shape[0]

    top2, wn = _route(x_flat, router_w)
    idxs, gates = [], []
    for e in range(E):
        sel = top2 == e
        we = np.where(sel, wn, 0.0).sum(-1)
        idx = np.nonzero(sel.any(-1))[0]
        idxs.append(idx)
        gates.append(we[idx].astype(np.float32))
    cap = max(len(i) for i in idxs)
    fastcap = ((cap + P - 1) // P) * P
    cap = ((cap + SCH - 1) // SCH) * SCH

    fast = bool(np.all(b1 == 0) and np.all(b2 == 0))
    if fast:
        cap = fastcap
    nc = _get_nc(cap, fast)

    in_maps = []
    for e in range(E):
        n_e = len(idxs[e])
        xg = np.zeros((cap, Cx), np.float32)
        xg[:n_e] = x_flat[idxs[e]]
        wg = np.zeros(cap, np.float32)
        wg[:n_e] = gates[e]
        if fast:
            import ml_dtypes

            bf16 = ml_dtypes.bfloat16
            xg *= wg[:, None]  # pre-gate: exact since b1 == 0 and wg >= 0
            sizes = _chunks(cap)
            blocks, off = [], 0
            for sz in sizes:
                blocks.append(
                    np.ascontiguousarray(
                        xg[off : off + sz].reshape(sz, 8, P).transpose(2, 1, 0)
                    )
                    .ravel()
                    .astype(bf16)
                )
                off += sz
            in_maps.append(
                {
                    "xgf": np.concatenate(blocks),
                    "w1p": np.ascontiguousarray(
                        w1[e].reshape(F // P, P, 8, P).transpose(0, 3, 2, 1)
                    ).astype(bf16),
                    "w2t": np.ascontiguousarray(w2[e].T).astype(bf16),
                }
            )
        else:
            in_maps.append(
                {
                    "xgT": np.ascontiguousarray(xg.T),
                    "w1t": np.ascontiguousarray(w1[e].T),
                    "w2t": np.ascontiguousarray(w2[e].T),
                    "b1r": np.ascontiguousarray(b1[e].reshape(F // P, P).T),
                    "b2r": np.ascontiguousarray(np.broadcast_to(b2[e], (P, Cx))),
                    "wg": np.ascontiguousarray(wg.reshape(cap // P, P).T),
                }
            )

    global _LAST_RES
    res = run_bass_kernel_spmd(nc, in_maps, core_ids=list(range(E)), trace=_TRACE)
    _LAST_RES = res

    out = np.zeros((T, Cx), np.float32)
    for e in range(E):
        n_e = len(idxs[e])
        out[idxs[e]] += res.results[e]["yg"][:n_e].astype(np.float32)
    return out.reshape(Bx, Nx, Cx)

